# revision 10
# baseline (speedup 1.0000x reference)
"""MultiHeadAttention Trainium2 kernel (8 NeuronCores).

Sharding: data-parallel over batch (2) x tensor-parallel over heads (16/4=4
head groups). Core c handles batch b = c//4 and heads 4g..4g+4 (g = c%4),
i.e. a 256-wide column slice of Wq/Wk/Wv and the matching row slice of Wo.
Each core computes a full [2048, 1024] partial output (its heads' ctx @ Wo
row-slice); the host sums the 4 partials per batch and adds the bias terms.

Per-core dataflow (all fp32 data, fp32r matmuls):
  xT       = transpose(x) via PE identity-matmuls      [d, s]
  Q.T, K.T = WqT.T @ xT + b (per-partition bias)       [d'=256, s] pair-packed
  V        = xT.T @ WvT (no bias; folded on host)      [s, c] + ones col/head
  scores.T = K.T_h.T @ Q.T_h (pairs via row tiles)     [k, q] in PSUM
  P.T      = exp(scores.T) on ACT (1/sqrt(dk) folded into Wq)
  ctx.T|r  = [V_h | 1].T @ P.T (M=65, rowsum for free) [65, q] PSUM accum
  ctx_n    = ctx.T * broadcast(1/r)                    [c, q] pair-packed
  out_u    = ctx_n.T @ WoT                             [s, 1024] -> DRAM
"""

import numpy as np

import concourse.bass as bass
import concourse.mybir as mybir
import concourse.tile as tile
from concourse import bacc
from concourse.bass_utils import run_bass_kernel_spmd

S = 2048          # sequence length
D = 1024          # model dim
DC = 256          # d' columns per core (4 heads x 64)
H = 4             # heads per core
DK = 64           # head dim
P = 128
F32 = mybir.dt.float32
F32R = mybir.dt.float32r
NCORES = 8

_cached = {}


def build_program():
    nc = bacc.Bacc("TRN2", target_bir_lowering=False, debug=False,
                   num_devices=NCORES)

    xq = nc.dram_tensor("xq", [S, D], F32R, kind="ExternalInput").ap()
    xk = nc.dram_tensor("xk", [S, D], F32R, kind="ExternalInput").ap()
    xv = nc.dram_tensor("xv", [S, D], F32R, kind="ExternalInput").ap()
    wqt = nc.dram_tensor("wqt", [D, DC], F32R, kind="ExternalInput").ap()
    wkt = nc.dram_tensor("wkt", [D, DC], F32R, kind="ExternalInput").ap()
    wvt = nc.dram_tensor("wvt", [D, DC], F32R, kind="ExternalInput").ap()
    wot = nc.dram_tensor("wot", [DC, D], F32R, kind="ExternalInput").ap()
    consts = nc.dram_tensor("consts", [P, P + 16], F32R,
                            kind="ExternalInput").ap()
    bqr = nc.dram_tensor("bqr", [2, P], F32, kind="ExternalInput").ap()
    bkr = nc.dram_tensor("bkr", [2, P], F32, kind="ExternalInput").ap()
    out = nc.dram_tensor("out", [S, D], F32, kind="ExternalOutput").ap()

    with tile.TileContext(nc) as tc:
        build_tile_kernel(nc, tc, xq, xk, xv, wqt, wkt, wvt, wot, consts,
                          bqr, bkr, out)

    nc.compile()
    return nc


def build_tile_kernel(nc, tc, xq, xk, xv, wqt, wkt, wvt, wot, consts,
                      bqr, bkr, out):
    from contextlib import ExitStack

    with ExitStack() as ctx:
        singles = ctx.enter_context(tc.tile_pool(name="singles", bufs=1))
        persist = ctx.enter_context(tc.tile_pool(name="persist", bufs=1))

        # --- constants / weights -------------------------------------------------
        ident = singles.tile([P, P], F32R)
        nc.sync.dma_start(out=ident, in_=consts[:, 0:P])

        w_q = singles.tile([P, 8, DC], F32R, tag="w_q")
        w_k = singles.tile([P, 8, DC], F32R, tag="w_k")
        w_v = singles.tile([P, 8, DC], F32R, tag="w_v")
        w_o = singles.tile([P, 2, D], F32R, tag="w_o")
        nc.sync.dma_start(out=w_q, in_=wqt.rearrange("(t p) c -> p t c", p=P))
        nc.sync.dma_start(out=w_k, in_=wkt.rearrange("(t p) c -> p t c", p=P))
        nc.sync.dma_start(out=w_v, in_=wvt.rearrange("(t p) c -> p t c", p=P))
        nc.sync.dma_start(out=w_o, in_=wot.rearrange("(t p) j -> p t j", p=P))

        bq_t = singles.tile([P, 2], F32, tag="bq")
        bk_t = singles.tile([P, 2], F32, tag="bk")
        nc.sync.dma_start(out=bq_t, in_=bqr.rearrange("m p -> p m"))
        nc.sync.dma_start(out=bk_t, in_=bkr.rearrange("m p -> p m"))

        # --- persistent activations ---------------------------------------------
        qT = persist.tile([P, 2, S], F32R, tag="qT")    # [d'%128, pair, s]
        kT = persist.tile([P, 2, S], F32R, tag="kT")
        v_sb = persist.tile([P, 16, H * (DK + 1)], F32R, tag="v_sb")
        ctxn = persist.tile([P, 2, S], F32R, tag="ctxn")  # [c%128, pair, q]

        for h in range(H):  # ones column per head for rowsum-in-matmul
            nc.sync.dma_start(out=v_sb[:, :, h * 65 + 64:h * 65 + 65],
                              in_=consts[:, P:P + 16].rearrange(
                                  "p (k o) -> p k o", o=1))

        # --- phase 1: transpose + projections ------------------------------------
        with tc.tile_pool(name="xin", bufs=2) as xin_pool, \
             tc.tile_pool(name="xT", bufs=2) as xT_pool, \
             tc.tile_pool(name="tp_ps", bufs=4, space="PSUM") as tp_ps, \
             tc.tile_pool(name="proj_ps", bufs=2, space="PSUM") as proj_ps, \
             tc.tile_pool(name="projv_ps", bufs=2, space="PSUM") as projv_ps:

            def transpose_chunk(x_dram, sc):
                """Load x[512sc:512sc+512, :] and produce xT [128, 8, 512]."""
                xin = xin_pool.tile([P, 4, D], F32R, tag="xin")
                nc.sync.dma_start(
                    out=xin,
                    in_=x_dram[512 * sc:512 * (sc + 1), :]
                        .rearrange("(si p) d -> p si d", p=P))
                xT = xT_pool.tile([P, 8, 512], F32R, tag="xT")
                for dt in range(8):
                    tp = tp_ps.tile([P, 4, P], F32R, tag="tp")
                    for si in range(4):
                        nc.tensor.matmul(
                            tp[:, si, :],
                            lhsT=xin[:, si, 128 * dt:128 * (dt + 1)],
                            rhs=ident,
                            is_transpose=True)
                    # evacuate on ACT (idle during phase 1)
                    nc.scalar.copy(xT[:, dt, :],
                                   tp.rearrange("p a b -> p (a b)"))
                return xT

            # K then Q (scores need both), then V
            for name, x_dram, w_t, b_t, dest in (
                    ("k", xk, w_k, bk_t, kT),
                    ("q", xq, w_q, bq_t, qT)):
                for sc in range(4):
                    xT = transpose_chunk(x_dram, sc)
                    for m in range(2):
                        pr = proj_ps.tile([P, 512], F32, tag="pr")
                        for dt in range(8):
                            nc.tensor.matmul(
                                pr,
                                lhsT=w_t[:, dt, 128 * m:128 * (m + 1)],
                                rhs=xT[:, dt, :],
                                start=(dt == 0), stop=(dt == 7))
                        nc.vector.tensor_scalar_add(
                            dest[:, m, 512 * sc:512 * (sc + 1)], pr,
                            b_t[:, m:m + 1])

            for sc in range(4):
                xT = transpose_chunk(xv, sc)
                for st in range(4):
                    pv = projv_ps.tile([P, DC], F32, tag="pv")
                    for dt in range(8):
                        nc.tensor.matmul(
                            pv,
                            lhsT=xT[:, dt, 128 * st:128 * (st + 1)],
                            rhs=w_v[:, dt, :],
                            start=(dt == 0), stop=(dt == 7))
                    kt = 4 * sc + st
                    nc.vector.tensor_copy(
                        v_sb[:, kt, :].rearrange("p (h c) -> p h c", h=H)[:, :, 0:DK],
                        pv.rearrange("p (h c) -> p h c", c=DK))

        # --- phase 2: attention ---------------------------------------------------
        with tc.tile_pool(name="sc_ps", bufs=2, space="PSUM") as sc_ps, \
             tc.tile_pool(name="ctx_ps", bufs=4, space="PSUM") as ctx_ps_pool, \
             tc.tile_pool(name="pT", bufs=4) as pT_pool, \
             tc.tile_pool(name="norm", bufs=4) as norm_pool:

            for qc in range(4):
                qsl = slice(512 * qc, 512 * (qc + 1))
                ctx_ps = [ctx_ps_pool.tile([P, 512], F32, tag="ctx",
                                           name=f"ctx_{qc}_{h}")
                          for h in range(H)]
                for kg in range(8):
                    for h in range(H):
                        pr, hp = divmod(h, 2)
                        base = 64 * hp
                        sc_t = sc_ps.tile([P, 2, 512], F32, tag="sc")
                        for khi in range(2):
                            kt = 2 * kg + khi
                            nc.tensor.matmul(
                                sc_t[:, khi, :],
                                lhsT=kT[base:base + 64, pr, 128 * kt:128 * (kt + 1)],
                                rhs=qT[base:base + 64, pr, qsl])
                        pT = pT_pool.tile([P, 2, 512], F32R, tag="pT")
                        nc.scalar.activation(
                            pT.rearrange("p a b -> p (a b)"),
                            sc_t.rearrange("p a b -> p (a b)"),
                            mybir.ActivationFunctionType.Exp)
                        for khi in range(2):
                            kt = 2 * kg + khi
                            nc.tensor.matmul(
                                ctx_ps[h][0:65, :],
                                lhsT=v_sb[:, kt, 65 * h:65 * h + 65],
                                rhs=pT[:, khi, :],
                                start=(kt == 0), stop=(kt == 15))
                # normalize: ctx_n = ctx * broadcast(1 / rowsum)
                for h in range(H):
                    pr, hp = divmod(h, 2)
                    base = 64 * hp
                    rec = norm_pool.tile([1, 512], F32, tag="rec")
                    nc.vector.reciprocal(rec, ctx_ps[h][64:65, :])
                    bc = norm_pool.tile([64, 512], F32, tag="bc")
                    nc.gpsimd.partition_broadcast(bc, rec[0:1, :], channels=64)
                    nc.vector.tensor_mul(
                        ctxn[base:base + 64, pr, qsl], ctx_ps[h][0:64, :], bc)

        # --- phase 3: output projection ------------------------------------------
        with tc.tile_pool(name="out_ps", bufs=2, space="PSUM") as out_ps, \
             tc.tile_pool(name="out_sb", bufs=2) as out_sb_pool:
            for st in range(16):
                ob = out_sb_pool.tile([P, D], F32, tag="ob")
                for jc in range(2):
                    op = out_ps.tile([P, 512], F32, tag="op")
                    for ct in range(2):
                        nc.tensor.matmul(
                            op,
                            lhsT=ctxn[:, ct, 128 * st:128 * (st + 1)],
                            rhs=w_o[:, ct, 512 * jc:512 * (jc + 1)],
                            start=(ct == 0), stop=(ct == 1))
                    nc.vector.tensor_copy(ob[:, 512 * jc:512 * (jc + 1)], op)
                nc.sync.dma_start(out=out[128 * st:128 * (st + 1), :], in_=ob)


def make_in_maps(Q_input, K_input, V_input, Wq, bq, Wk, bk, Wv, Wo):
    scale = 0.125  # 1/sqrt(64), exact power of two
    consts = np.concatenate(
        [np.eye(P, dtype=np.float32),
         np.ones((P, 16), np.float32)], axis=1)
    in_maps = []
    for c in range(NCORES):
        b, g = divmod(c, 4)
        sl = slice(DC * g, DC * (g + 1))
        in_maps.append({
            "xq": np.ascontiguousarray(Q_input[b]),
            "xk": np.ascontiguousarray(K_input[b]),
            "xv": np.ascontiguousarray(V_input[b]),
            "wqt": np.ascontiguousarray(Wq[sl, :].T) * scale,
            "wkt": np.ascontiguousarray(Wk[sl, :].T),
            "wvt": np.ascontiguousarray(Wv[sl, :].T),
            "wot": np.ascontiguousarray(Wo[:, sl].T),
            "consts": consts,
            "bqr": (bq[sl] * scale).reshape(2, P).copy(),
            "bkr": bk[sl].reshape(2, P).copy(),
        })
    return in_maps


def kernel(Q_input, K_input, V_input, Wq, bq, Wk, bk, Wv, bv, Wo, bo):
    if "nc" not in _cached:
        _cached["nc"] = build_program()
    nc = _cached["nc"]

    in_maps = make_in_maps(Q_input, K_input, V_input, Wq, bq, Wk, bk, Wv, Wo)
    res = run_bass_kernel_spmd(nc, in_maps, list(range(NCORES))).results
    outs = [res[c]["out"] for c in range(NCORES)]

    const = (bv.astype(np.float32) @ Wo.T.astype(np.float32)) + bo
    full = np.empty((2, S, D), np.float32)
    for b in range(2):
        acc = outs[4 * b].astype(np.float32).copy()
        for g in range(1, 4):
            acc += outs[4 * b + g]
        full[b] = acc + const
    return full


# revision 11
# speedup vs baseline: 1.4618x; 1.4618x over previous
"""MultiHeadAttention Trainium2 kernel (8 NeuronCores).

Sharding: data-parallel over batch (2) x tensor-parallel over heads (16/4=4
head groups). Core c handles batch b = c//4 and heads 4g..4g+4 (g = c%4),
i.e. a 256-wide column slice of Wq/Wk/Wv and the matching row slice of Wo.
Each core computes a full [2048, 1024] partial output (its heads' ctx @ Wo
row-slice); the host sums the 4 partials per batch and adds the bias terms.

v2: fp16 on-chip datapath. The host supplies x already transposed and cast
to fp16 (xT [1024, 2048]), so no on-chip transposes are needed. All matmul
operands are fp16 (fp32 PSUM accumulation), which enables fast weight load
and back-to-back matmul pipelining. 1/sqrt(dk) is folded into Wq/bq.

Per-core dataflow:
  Q.T, K.T = W.T @ xT + b (per-partition bias)     [d'=256, s] pair-packed
  V        = xT.T @ WvT (no bias; folded on host)  [s, c] + ones col/head
  scores.T = K.T_h.T @ Q.T_h (row-tile head pairs) [k, q] in PSUM
  P.T      = exp(scores.T) on ACT, fp16            [k, q] SBUF
  ctx.T|r  = [V_h | 1].T @ P.T (M=65, fused rowsum), 4-k-tile partial
             chains in PSUM summed into f32 SBUF accumulators
  ctx_n    = ctx.T * broadcast(1/r)                [c, q] pair-packed fp16
  out_u    = ctx_n.T @ WoT                         [s, 1024] -> DRAM f32
"""

import numpy as np

import concourse.bass as bass
import concourse.mybir as mybir
import concourse.tile as tile
from concourse import bacc
from concourse.bass_utils import run_bass_kernel_spmd

S = 2048          # sequence length
D = 1024          # model dim
DC = 256          # d' columns per core (4 heads x 64)
H = 4             # heads per core
DK = 64           # head dim
P = 128
F32 = mybir.dt.float32
FP16 = mybir.dt.float16
NCORES = 8

_cached = {}


def build_program():
    nc = bacc.Bacc("TRN2", target_bir_lowering=False, debug=False,
                   num_devices=NCORES)

    xqT = nc.dram_tensor("xqT", [D, S], FP16, kind="ExternalInput").ap()
    xkT = nc.dram_tensor("xkT", [D, S], FP16, kind="ExternalInput").ap()
    xvT = nc.dram_tensor("xvT", [D, S], FP16, kind="ExternalInput").ap()
    wqt = nc.dram_tensor("wqt", [D, DC], FP16, kind="ExternalInput").ap()
    wkt = nc.dram_tensor("wkt", [D, DC], FP16, kind="ExternalInput").ap()
    wvt = nc.dram_tensor("wvt", [D, DC], FP16, kind="ExternalInput").ap()
    wot = nc.dram_tensor("wot", [DC, D], FP16, kind="ExternalInput").ap()
    ones16 = nc.dram_tensor("ones16", [P, 16], FP16, kind="ExternalInput").ap()
    bqr = nc.dram_tensor("bqr", [2, P], F32, kind="ExternalInput").ap()
    bkr = nc.dram_tensor("bkr", [2, P], F32, kind="ExternalInput").ap()
    out = nc.dram_tensor("out", [S, D], F32, kind="ExternalOutput").ap()

    with tile.TileContext(nc) as tc:
        build_tile_kernel(nc, tc, xqT, xkT, xvT, wqt, wkt, wvt, wot, ones16,
                          bqr, bkr, out)

    nc.compile()
    return nc


def build_tile_kernel(nc, tc, xqT, xkT, xvT, wqt, wkt, wvt, wot, ones16,
                      bqr, bkr, out):
    from contextlib import ExitStack

    with ExitStack() as ctx:
        singles = ctx.enter_context(tc.tile_pool(name="singles", bufs=1))
        persist = ctx.enter_context(tc.tile_pool(name="persist", bufs=1))
        # shared PSUM pools for ALL phases (so phases can overlap):
        #  psA: 2-bank slots (scores tiles), psB: 1-bank slots (everything else)
        psA = ctx.enter_context(tc.tile_pool(name="psA", bufs=2, space="PSUM"))
        psB = ctx.enter_context(tc.tile_pool(name="psB", bufs=4, space="PSUM"))
        xT_pool = ctx.enter_context(tc.tile_pool(name="xT", bufs=2))
        pT_pool = ctx.enter_context(tc.tile_pool(name="pT", bufs=4))
        acc_pool = ctx.enter_context(tc.tile_pool(name="acc", bufs=8))
        norm_pool = ctx.enter_context(tc.tile_pool(name="norm", bufs=4))
        out_sb_pool = ctx.enter_context(tc.tile_pool(name="osb", bufs=3))

        # --- constants / weights -------------------------------------------------
        w_q = singles.tile([P, 8, DC], FP16, tag="w_q")
        w_k = singles.tile([P, 8, DC], FP16, tag="w_k")
        w_v = singles.tile([P, 8, DC], FP16, tag="w_v")
        w_o = singles.tile([P, 2, D], FP16, tag="w_o")
        nc.sync.dma_start(out=w_q, in_=wqt.rearrange("(t p) c -> p t c", p=P))
        nc.sync.dma_start(out=w_k, in_=wkt.rearrange("(t p) c -> p t c", p=P))
        nc.sync.dma_start(out=w_v, in_=wvt.rearrange("(t p) c -> p t c", p=P))
        nc.sync.dma_start(out=w_o, in_=wot.rearrange("(t p) j -> p t j", p=P))

        bq_t = singles.tile([P, 2], F32, tag="bq")
        bk_t = singles.tile([P, 2], F32, tag="bk")
        nc.sync.dma_start(out=bq_t, in_=bqr.rearrange("m p -> p m"))
        nc.sync.dma_start(out=bk_t, in_=bkr.rearrange("m p -> p m"))

        # --- persistent activations ---------------------------------------------
        qT = persist.tile([P, 2, S], FP16, tag="qT")    # [d'%128, pair, s]
        kT = persist.tile([P, 2, S], FP16, tag="kT")
        v_sb = persist.tile([P, 16, H * (DK + 1)], FP16, tag="v_sb")
        ctxn = persist.tile([P, 2, S], FP16, tag="ctxn")  # [c%128, pair, q]

        for h in range(H):  # ones column per head for rowsum-in-matmul
            nc.sync.dma_start(out=v_sb[:, :, h * 65 + 64:h * 65 + 65],
                              in_=ones16.rearrange("p (k o) -> p k o", o=1))

        # --- phase 1: projections (xT comes pre-transposed from host) -----------
        def load_xT_chunk(x_dram, sc):
            xc = xT_pool.tile([P, 8, 512], FP16, tag="xc", name=f"xc_{sc}")
            nc.sync.dma_start(
                out=xc,
                in_=x_dram[:, 512 * sc:512 * (sc + 1)]
                    .rearrange("(t p) s -> p t s", p=P))
            return xc

        # K then Q (scores need both), then V
        for name, x_dram, w_t, b_t, dest in (
                ("k", xkT, w_k, bk_t, kT),
                ("q", xqT, w_q, bq_t, qT)):
            for sc in range(4):
                xc = load_xT_chunk(x_dram, sc)
                for m in range(2):
                    pr = psB.tile([P, 512], F32, tag="ps1",
                                  name=f"pr_{name}_{sc}_{m}")
                    for dt in range(8):
                        nc.tensor.matmul(
                            pr,
                            lhsT=w_t[:, dt, 128 * m:128 * (m + 1)],
                            rhs=xc[:, dt, :],
                            start=(dt == 0), stop=(dt == 7))
                    nc.vector.tensor_scalar_add(
                        dest[:, m, 512 * sc:512 * (sc + 1)], pr,
                        b_t[:, m:m + 1])

        for sc in range(4):
            xc = load_xT_chunk(xvT, sc)
            for st in range(4):
                pv = psB.tile([P, DC], F32, tag="ps1", name=f"pv_{sc}_{st}")
                for dt in range(8):
                    nc.tensor.matmul(
                        pv,
                        lhsT=xc[:, dt, 128 * st:128 * (st + 1)],
                        rhs=w_v[:, dt, :],
                        start=(dt == 0), stop=(dt == 7))
                kt = 4 * sc + st
                nc.vector.tensor_copy(
                    v_sb[:, kt, :].rearrange("p (h c) -> p h c", h=H)[:, :, 0:DK],
                    pv.rearrange("p (h c) -> p h c", c=DK))

        # --- phase 2: attention + per-qc output projection ------------------------
        for qc in range(4):
            qsl = slice(512 * qc, 512 * (qc + 1))
            for pr_i in range(2):
                h_a, h_b = 2 * pr_i, 2 * pr_i + 1
                pT_a = pT_pool.tile([P, 16, 512], FP16, tag="pT",
                                    name=f"pT_{qc}_{h_a}")
                pT_b = pT_pool.tile([P, 16, 512], FP16, tag="pT",
                                    name=f"pT_{qc}_{h_b}")
                # scores + exp, paired across the two heads (row groups 0/64)
                for kg in range(8):
                    sc_a = psA.tile([P, 2, 512], F32, tag="sc",
                                    name=f"sca_{qc}_{pr_i}_{kg}")
                    sc_b = psA.tile([P, 2, 512], F32, tag="sc",
                                    name=f"scb_{qc}_{pr_i}_{kg}")
                    for khi in range(2):
                        kt = 2 * kg + khi
                        ksl = slice(128 * kt, 128 * (kt + 1))
                        nc.tensor.matmul(sc_a[:, khi, :],
                                         lhsT=kT[0:64, pr_i, ksl],
                                         rhs=qT[0:64, pr_i, qsl])
                        nc.tensor.matmul(sc_b[:, khi, :],
                                         lhsT=kT[64:128, pr_i, ksl],
                                         rhs=qT[64:128, pr_i, qsl])
                    nc.scalar.activation(
                        pT_a[:, 2 * kg:2 * kg + 2, :].rearrange(
                            "p a b -> p (a b)"),
                        sc_a.rearrange("p a b -> p (a b)"),
                        mybir.ActivationFunctionType.Exp)
                    nc.scalar.activation(
                        pT_b[:, 2 * kg:2 * kg + 2, :].rearrange(
                            "p a b -> p (a b)"),
                        sc_b.rearrange("p a b -> p (a b)"),
                        mybir.ActivationFunctionType.Exp)
                # ctx: 4-k-tile partial chains in PSUM, accumulated in SBUF f32
                for h, pT_h in ((h_a, pT_a), (h_b, pT_b)):
                    acc = acc_pool.tile([65, 512], F32, tag="acc",
                                        name=f"acc_{qc}_{h}")
                    for gg in range(4):
                        cp = psB.tile([P, 512], F32, tag="ps1",
                                      name=f"cp_{qc}_{h}_{gg}")
                        for j in range(4):
                            kt = 4 * gg + j
                            nc.tensor.matmul(
                                cp[0:65, :],
                                lhsT=v_sb[:, kt, 65 * h:65 * h + 65],
                                rhs=pT_h[:, kt, :],
                                start=(j == 0), stop=(j == 3))
                        if gg == 0:
                            nc.vector.tensor_copy(acc, cp[0:65, :])
                        else:
                            nc.vector.tensor_add(acc, acc, cp[0:65, :])
                    # normalize: ctx_n = ctx * broadcast(1/rowsum)
                    rec = norm_pool.tile([1, 512], F32, tag="rec",
                                         name=f"rec_{qc}_{h}")
                    nc.vector.reciprocal(rec, acc[64:65, :])
                    bc = norm_pool.tile([64, 512], F32, tag="bc",
                                        name=f"bc_{qc}_{h}")
                    nc.gpsimd.partition_broadcast(bc, rec[0:1, :], channels=64)
                    hp = h % 2
                    nc.vector.tensor_mul(
                        ctxn[64 * hp:64 * hp + 64, pr_i, qsl],
                        acc[0:64, :], bc)
            # output projection for this q-chunk's 4 s-tiles
            for st in range(4 * qc, 4 * qc + 4):
                ob = out_sb_pool.tile([P, D], F32, tag="ob", name=f"ob_{st}")
                for jc in range(2):
                    op = psB.tile([P, 512], F32, tag="ps1", name=f"op_{st}_{jc}")
                    for ct in range(2):
                        nc.tensor.matmul(
                            op,
                            lhsT=ctxn[:, ct, 128 * st:128 * (st + 1)],
                            rhs=w_o[:, ct, 512 * jc:512 * (jc + 1)],
                            start=(ct == 0), stop=(ct == 1))
                    nc.vector.tensor_copy(ob[:, 512 * jc:512 * (jc + 1)], op)
                nc.sync.dma_start(out=out[128 * st:128 * (st + 1), :], in_=ob)


def make_in_maps(Q_input, K_input, V_input, Wq, bq, Wk, bk, Wv, Wo):
    scale = 0.125  # 1/sqrt(64), exact power of two
    ones16 = np.ones((P, 16), np.float16)
    xT16 = {}
    for b in range(2):
        xT16[("q", b)] = np.ascontiguousarray(Q_input[b].T).astype(np.float16)
        xT16[("k", b)] = np.ascontiguousarray(K_input[b].T).astype(np.float16)
        xT16[("v", b)] = np.ascontiguousarray(V_input[b].T).astype(np.float16)
    in_maps = []
    for c in range(NCORES):
        b, g = divmod(c, 4)
        sl = slice(DC * g, DC * (g + 1))
        in_maps.append({
            "xqT": xT16[("q", b)],
            "xkT": xT16[("k", b)],
            "xvT": xT16[("v", b)],
            "wqt": (np.ascontiguousarray(Wq[sl, :].T) * scale).astype(np.float16),
            "wkt": np.ascontiguousarray(Wk[sl, :].T).astype(np.float16),
            "wvt": np.ascontiguousarray(Wv[sl, :].T).astype(np.float16),
            "wot": np.ascontiguousarray(Wo[:, sl].T).astype(np.float16),
            "ones16": ones16,
            "bqr": (bq[sl] * scale).reshape(2, P).astype(np.float32),
            "bkr": bk[sl].reshape(2, P).astype(np.float32),
        })
    return in_maps


def kernel(Q_input, K_input, V_input, Wq, bq, Wk, bk, Wv, bv, Wo, bo):
    if "nc" not in _cached:
        _cached["nc"] = build_program()
    nc = _cached["nc"]

    in_maps = make_in_maps(Q_input, K_input, V_input, Wq, bq, Wk, bk, Wv, Wo)
    res = run_bass_kernel_spmd(nc, in_maps, list(range(NCORES))).results
    outs = [res[c]["out"] for c in range(NCORES)]

    const = (bv.astype(np.float32) @ Wo.T.astype(np.float32)) + bo
    full = np.empty((2, S, D), np.float32)
    for b in range(2):
        acc = outs[4 * b].astype(np.float32).copy()
        for g in range(1, 4):
            acc += outs[4 * b + g]
        full[b] = acc + const
    return full


# revision 17
# speedup vs baseline: 1.5153x; 1.0366x over previous
"""MultiHeadAttention Trainium2 kernel (8 NeuronCores).

Sharding: data-parallel over batch (2) x tensor-parallel over heads (16/4=4
head groups). Core c handles batch b = c//4 and heads 4g..4g+4 (g = c%4),
i.e. a 256-wide column slice of Wq/Wk/Wv and the matching row slice of Wo.
Each core computes a full [2048, 1024] partial output (its heads' ctx @ Wo
row-slice); the host sums the 4 partials per batch and adds the bias terms.

v2: fp16 on-chip datapath. The host supplies x already transposed and cast
to fp16 (xT [1024, 2048]), so no on-chip transposes are needed. All matmul
operands are fp16 (fp32 PSUM accumulation), which enables fast weight load
and back-to-back matmul pipelining. 1/sqrt(dk) is folded into Wq/bq.

Per-core dataflow:
  Q.T, K.T = W.T @ xT + b (per-partition bias)     [d'=256, s] pair-packed
  V        = xT.T @ WvT (no bias; folded on host)  [s, c] + ones col/head
  scores.T = K.T_h.T @ Q.T_h (row-tile head pairs) [k, q] in PSUM
  P.T      = exp(scores.T) on ACT, fp16            [k, q] SBUF
  ctx.T|r  = [V_h | 1].T @ P.T (M=65, fused rowsum), 4-k-tile partial
             chains in PSUM summed into f32 SBUF accumulators
  ctx_n    = ctx.T * broadcast(1/r)                [c, q] pair-packed fp16
  out_u    = ctx_n.T @ WoT                         [s, 1024] -> DRAM f32
"""

import numpy as np

import concourse.bass as bass
import concourse.mybir as mybir
import concourse.tile as tile
from concourse import bacc
from concourse.bass_utils import run_bass_kernel_spmd

S = 2048          # sequence length
D = 1024          # model dim
DC = 256          # d' columns per core (4 heads x 64)
H = 4             # heads per core
DK = 64           # head dim
P = 128
F32 = mybir.dt.float32
FP16 = mybir.dt.float16
NCORES = 8

_cached = {}


def build_program():
    nc = bacc.Bacc("TRN2", target_bir_lowering=False, debug=False,
                   num_devices=NCORES)

    xqT = nc.dram_tensor("xqT", [D, S], FP16, kind="ExternalInput").ap()
    xkT = nc.dram_tensor("xkT", [D, S], FP16, kind="ExternalInput").ap()
    xvT = nc.dram_tensor("xvT", [D, S], FP16, kind="ExternalInput").ap()
    wqt = nc.dram_tensor("wqt", [D, DC], FP16, kind="ExternalInput").ap()
    wkt = nc.dram_tensor("wkt", [D, DC], FP16, kind="ExternalInput").ap()
    wvt = nc.dram_tensor("wvt", [D, DC], FP16, kind="ExternalInput").ap()
    wot = nc.dram_tensor("wot", [DC, D], FP16, kind="ExternalInput").ap()
    ones16 = nc.dram_tensor("ones16", [P, 16], FP16,
                            kind="ExternalInput").ap()
    bqr = nc.dram_tensor("bqr", [2, P], F32, kind="ExternalInput").ap()
    bkr = nc.dram_tensor("bkr", [2, P], F32, kind="ExternalInput").ap()
    out = nc.dram_tensor("out", [S, D], F32, kind="ExternalOutput").ap()

    with tile.TileContext(nc) as tc:
        build_tile_kernel(nc, tc, xqT, xkT, xvT, wqt, wkt, wvt, wot,
                          ones16, bqr, bkr, out)

    nc.compile()
    return nc


def build_tile_kernel(nc, tc, xqT, xkT, xvT, wqt, wkt, wvt, wot,
                      ones16, bqr, bkr, out):
    from contextlib import ExitStack

    with ExitStack() as ctx:
        singles = ctx.enter_context(tc.tile_pool(name="singles", bufs=1))
        persist = ctx.enter_context(tc.tile_pool(name="persist", bufs=1))
        # shared PSUM pools for ALL phases (so phases can overlap):
        #  psA: 2-bank slots (scores tiles), psB: 1-bank slots (everything else)
        psA = ctx.enter_context(tc.tile_pool(name="psA", bufs=2, space="PSUM"))
        psB = ctx.enter_context(tc.tile_pool(name="psB", bufs=4, space="PSUM"))
        xT_pool = ctx.enter_context(tc.tile_pool(name="xT", bufs=4))
        pT_pool = ctx.enter_context(tc.tile_pool(name="pT", bufs=4))
        acc_pool = ctx.enter_context(tc.tile_pool(name="acc", bufs=8))
        norm_pool = ctx.enter_context(tc.tile_pool(name="norm", bufs=4))
        out_sb_pool = ctx.enter_context(tc.tile_pool(name="osb", bufs=3))

        # --- constants / weights -------------------------------------------------
        w_q = singles.tile([P, 8, DC], FP16, tag="w_q")
        w_k = singles.tile([P, 8, DC], FP16, tag="w_k")
        w_v = singles.tile([P, 8, DC], FP16, tag="w_v")
        w_o = singles.tile([P, 2, D], FP16, tag="w_o")
        nc.sync.dma_start(out=w_q, in_=wqt.rearrange("(t p) c -> p t c", p=P))
        nc.sync.dma_start(out=w_k, in_=wkt.rearrange("(t p) c -> p t c", p=P))
        nc.sync.dma_start(out=w_v, in_=wvt.rearrange("(t p) c -> p t c", p=P))
        nc.sync.dma_start(out=w_o, in_=wot.rearrange("(t p) j -> p t j", p=P))

        bq_t = singles.tile([P, 2], F32, tag="bq")
        bk_t = singles.tile([P, 2], F32, tag="bk")
        nc.sync.dma_start(out=bq_t, in_=bqr.rearrange("m p -> p m"))
        nc.sync.dma_start(out=bk_t, in_=bkr.rearrange("m p -> p m"))

        # --- persistent activations ---------------------------------------------
        qT = persist.tile([P, 2, S], FP16, tag="qT")    # [d'%128, pair, s]
        kT = persist.tile([P, 2, S], FP16, tag="kT")
        v_sb = persist.tile([P, 16, H * (DK + 1)], FP16, tag="v_sb")
        ctxn = persist.tile([P, 2, S], FP16, tag="ctxn")  # [c%128, pair, q]

        for h in range(H):  # ones column per head for rowsum-in-matmul
            nc.sync.dma_start(out=v_sb[:, :, h * 65 + 64:h * 65 + 65],
                              in_=ones16.rearrange("p (k o) -> p k o", o=1))

        # --- phase 1: projections (xT comes pre-transposed from host) -----------
        def load_xT_chunk(x_dram, sc):
            xc = xT_pool.tile([P, 8, 512], FP16, tag="xc", name=f"xc_{sc}")
            nc.sync.dma_start(
                out=xc,
                in_=x_dram[:, 512 * sc:512 * (sc + 1)]
                    .rearrange("(t p) s -> p t s", p=P))
            return xc

        # K then Q (scores need both), then V
        for name, x_dram, w_t, b_t, dest in (
                ("k", xkT, w_k, bk_t, kT),
                ("q", xqT, w_q, bq_t, qT)):
            for sc in range(4):
                xc = load_xT_chunk(x_dram, sc)
                for m in range(2):
                    pr = psB.tile([P, 512], F32, tag="ps1",
                                  name=f"pr_{name}_{sc}_{m}")
                    for dt in range(8):
                        nc.tensor.matmul(
                            pr,
                            lhsT=w_t[:, dt, 128 * m:128 * (m + 1)],
                            rhs=xc[:, dt, :],
                            start=(dt == 0), stop=(dt == 7))
                    nc.vector.tensor_scalar_add(
                        dest[:, m, 512 * sc:512 * (sc + 1)], pr,
                        b_t[:, m:m + 1])

        for sc in range(4):
            xc = load_xT_chunk(xvT, sc)
            for st in range(4):
                pv = psB.tile([P, DC], F32, tag="ps1", name=f"pv_{sc}_{st}")
                for dt in range(8):
                    nc.tensor.matmul(
                        pv,
                        lhsT=xc[:, dt, 128 * st:128 * (st + 1)],
                        rhs=w_v[:, dt, :],
                        start=(dt == 0), stop=(dt == 7))
                kt = 4 * sc + st
                nc.vector.tensor_copy(
                    v_sb[:, kt, :].rearrange("p (h c) -> p h c", h=H)[:, :, 0:DK],
                    pv.rearrange("p (h c) -> p h c", c=DK))

        # --- phase 2: attention + per-qc output projection ------------------------
        for qc in range(4):
            qsl = slice(512 * qc, 512 * (qc + 1))
            for pr_i in range(2):
                h_a, h_b = 2 * pr_i, 2 * pr_i + 1
                pT_a = pT_pool.tile([P, 16, 512], FP16, tag="pT",
                                    name=f"pT_{qc}_{h_a}")
                pT_b = pT_pool.tile([P, 16, 512], FP16, tag="pT",
                                    name=f"pT_{qc}_{h_b}")
                # scores + exp, paired across the two heads (row groups 0/64)
                for kg in range(8):
                    sc_a = psA.tile([P, 2, 512], F32, tag="sc",
                                    name=f"sca_{qc}_{pr_i}_{kg}")
                    sc_b = psA.tile([P, 2, 512], F32, tag="sc",
                                    name=f"scb_{qc}_{pr_i}_{kg}")
                    for khi in range(2):
                        kt = 2 * kg + khi
                        ksl = slice(128 * kt, 128 * (kt + 1))
                        nc.tensor.matmul(sc_a[:, khi, :],
                                         lhsT=kT[0:64, pr_i, ksl],
                                         rhs=qT[0:64, pr_i, qsl])
                        nc.tensor.matmul(sc_b[:, khi, :],
                                         lhsT=kT[64:128, pr_i, ksl],
                                         rhs=qT[64:128, pr_i, qsl])
                    nc.scalar.activation(
                        pT_a[:, 2 * kg:2 * kg + 2, :].rearrange(
                            "p a b -> p (a b)"),
                        sc_a.rearrange("p a b -> p (a b)"),
                        mybir.ActivationFunctionType.Exp)
                    nc.scalar.activation(
                        pT_b[:, 2 * kg:2 * kg + 2, :].rearrange(
                            "p a b -> p (a b)"),
                        sc_b.rearrange("p a b -> p (a b)"),
                        mybir.ActivationFunctionType.Exp)
                # ctx: 4-k-tile partial chains in PSUM, accumulated in SBUF f32
                for h, pT_h in ((h_a, pT_a), (h_b, pT_b)):
                    acc = acc_pool.tile([65, 512], F32, tag="acc",
                                        name=f"acc_{qc}_{h}")
                    for gg in range(4):
                        cp = psB.tile([P, 512], F32, tag="ps1",
                                      name=f"cp_{qc}_{h}_{gg}")
                        for j in range(4):
                            kt = 4 * gg + j
                            nc.tensor.matmul(
                                cp[0:65, :],
                                lhsT=v_sb[:, kt, 65 * h:65 * h + 65],
                                rhs=pT_h[:, kt, :],
                                start=(j == 0), stop=(j == 3))
                        if gg == 0:
                            nc.vector.tensor_copy(acc, cp[0:65, :])
                        else:
                            nc.vector.tensor_add(acc, acc, cp[0:65, :])
                    # normalize: ctx_n = ctx * broadcast(1/rowsum)
                    rs = norm_pool.tile([1, 512], F32, tag="rs",
                                        name=f"rs_{qc}_{h}")
                    nc.vector.tensor_copy(rs, acc[64:65, :])
                    bc = norm_pool.tile([64, 512], F32, tag="bc",
                                        name=f"bc_{qc}_{h}")
                    nc.gpsimd.partition_broadcast(bc, rs[0:1, :], channels=64)
                    rc = norm_pool.tile([64, 512], F32, tag="rc",
                                        name=f"rc_{qc}_{h}")
                    nc.vector.reciprocal(rc, bc)
                    hp = h % 2
                    nc.vector.tensor_mul(
                        ctxn[64 * hp:64 * hp + 64, pr_i, qsl],
                        acc[0:64, :], rc)
            # output projection for this q-chunk's 4 s-tiles
            for st in range(4 * qc, 4 * qc + 4):
                ob = out_sb_pool.tile([P, D], F32, tag="ob", name=f"ob_{st}")
                for jc in range(2):
                    op = psB.tile([P, 512], F32, tag="ps1", name=f"op_{st}_{jc}")
                    for ct in range(2):
                        nc.tensor.matmul(
                            op,
                            lhsT=ctxn[:, ct, 128 * st:128 * (st + 1)],
                            rhs=w_o[:, ct, 512 * jc:512 * (jc + 1)],
                            start=(ct == 0), stop=(ct == 1))
                    nc.vector.tensor_copy(ob[:, 512 * jc:512 * (jc + 1)], op)
                nc.sync.dma_start(out=out[128 * st:128 * (st + 1), :], in_=ob)


def make_in_maps(Q_input, K_input, V_input, Wq, bq, Wk, bk, Wv, Wo):
    scale = 0.125  # 1/sqrt(64), exact power of two
    xT16 = {}
    for b in range(2):
        xT16[("q", b)] = np.ascontiguousarray(Q_input[b].T).astype(np.float16)
        xT16[("k", b)] = np.ascontiguousarray(K_input[b].T).astype(np.float16)
        xT16[("v", b)] = np.ascontiguousarray(V_input[b].T).astype(np.float16)
    ones16 = np.ones((P, 16), np.float16)
    in_maps = []
    for c in range(NCORES):
        b, g = divmod(c, 4)
        sl = slice(DC * g, DC * (g + 1))
        in_maps.append({
            "xqT": xT16[("q", b)],
            "xkT": xT16[("k", b)],
            "xvT": xT16[("v", b)],
            "wqt": (np.ascontiguousarray(Wq[sl, :].T) * scale).astype(np.float16),
            "wkt": np.ascontiguousarray(Wk[sl, :].T).astype(np.float16),
            "wvt": np.ascontiguousarray(Wv[sl, :].T).astype(np.float16),
            "wot": np.ascontiguousarray(Wo[:, sl].T).astype(np.float16),
            "ones16": ones16,
            "bqr": (bq[sl] * scale).reshape(2, P).astype(np.float32),
            "bkr": bk[sl].reshape(2, P).astype(np.float32),
        })
    return in_maps


def kernel(Q_input, K_input, V_input, Wq, bq, Wk, bk, Wv, bv, Wo, bo):
    if "nc" not in _cached:
        _cached["nc"] = build_program()
    nc = _cached["nc"]

    in_maps = make_in_maps(Q_input, K_input, V_input, Wq, bq, Wk, bk, Wv, Wo)
    res = run_bass_kernel_spmd(nc, in_maps, list(range(NCORES))).results
    outs = [res[c]["out"] for c in range(NCORES)]

    const = (bv.astype(np.float32) @ Wo.T.astype(np.float32)) + bo
    full = np.empty((2, S, D), np.float32)
    for b in range(2):
        acc = outs[4 * b].astype(np.float32).copy()
        for g in range(1, 4):
            acc += outs[4 * b + g]
        full[b] = acc + const
    return full
